# revision 2
# baseline (speedup 1.0000x reference)
"""Trainium2 Bass kernel for a binarized (1w1a) BasicBlock:

    x1  = BN1(PReLU(conv3x3(sign(x),  std*sign(W1)) + b1)) + x
    out = BN2(PReLU(conv3x3(sign(x1), std*sign(W2)) + b2)) + x1

Data-parallel over the batch axis: each of the 8 NeuronCores processes 8 of
the 64 images (weights / BN / PReLU params replicated; no collectives).

Per-core design (HW-measured at ~158 us vs 165 us for the previous version):
  * Binarized activations/weights live in fp8 strips (per-image, zero-padded
    33x33 planes with guard bands); the 3x3 conv is 9 shifted matmuls
    accumulating in PSUM using the fp8 DoubleRow perf mode (K=256/pass,
    157 TF/s).  Windows are half-image (16 rows x 32 real cols = 512
    outputs = exactly one PSUM bank) addressed with 4-D strided rhs APs
    [128, 2, 16, 32], so no PE cycles are spent on pad rows/columns.
  * PSUM evacuation is ONE ScalarE op: t = Prelu(S*sc + g*b; alpha) -- the
    conv bias, BN scale and PReLU fold into the activation (g>0 commutes
    with PReLU).  One VectorE op then adds the BN shift + residual.
  * sign(x)/sign(out1) via VectorE (v>0)-0.5 / ScalarE Sign; weight
    binarize on VectorE (GpSimd ALU is ~18x slower and throttles VectorE).
  * Startup: PE p-state warm-up matmuls on zeroed tiles; startup-critical
    DMAs (first weight half in 3 chunks + a bf16 sign-only copy of image 0)
    are protected from bandwidth sharing by serialization gates on the
    scalar queue; per-image tiles keep Tile deps fine-grained.
  * Tail: the last image's second half runs as 8-row windows so the final
    evac/add/DMA chain is short.  Output is written as bf16 (host upcasts).
"""

import math
import os
import sys

import numpy as np
import ml_dtypes

for _p in ("/opt/trn_rl_repo", "/root/.axon_site/_ro/trn_rl_repo"):
    if os.path.isdir(_p) and _p not in sys.path:
        sys.path.insert(0, _p)

import concourse.bass as bass
import concourse.bacc as bacc
import concourse.mybir as mybir
from concourse import tile
from concourse.bass_utils import run_bass_kernel_spmd

F32 = mybir.dt.float32
BF16 = mybir.dt.bfloat16
F8 = mybir.dt.float8e4
U32 = mybir.dt.uint32
AOP = mybir.AluOpType
AFT = mybir.ActivationFunctionType
DR = mybir.MatmulPerfMode.DoubleRow

EPS = 1e-5
NCORES = 8
NIMG = 8              # images per core
NROW = 33             # strip rows per image plane (32 real + 1 pad)
NCOL = 33             # strip cols per plane (32 real + 1 pad)
PLANE = NROW * NCOL   # 1089
GF = 48               # front guard (>= 34)
GB = 15               # back guard; GF + PLANE + GB = 1152, 16-aligned
SLEN = GF + PLANE + GB
IMGE = 1024           # compact elements per image per half (32*32)
STD = math.sqrt(2.0) / math.sqrt(256 * 9)

PARAM_ORDER = [
    "b1", "alpha", "bn1_gamma", "bn1_beta", "bn1_mean", "bn1_var",
    "b2", "bn2_gamma", "bn2_beta", "bn2_mean", "bn2_var",
]
NPARAM = len(PARAM_ORDER)


def build_program():
    nc = bacc.Bacc("TRN2", target_bir_lowering=False, debug=False,
                   num_devices=NCORES)

    xs = nc.declare_dram_parameter("xs", [2, 128, NIMG * IMGE], F32,
                                   isOutput=False)
    x0b = nc.declare_dram_parameter("x0b", [2, 128, IMGE], BF16,
                                    isOutput=False)
    w1 = nc.declare_dram_parameter("w1", [2, 128, 9, 2, 128], BF16,
                                   isOutput=False)
    w2 = nc.declare_dram_parameter("w2", [2, 128, 9, 2, 128], BF16,
                                   isOutput=False)
    pv = nc.declare_dram_parameter("pv", [128, 2 * NPARAM], F32,
                                   isOutput=False)
    outd = nc.declare_dram_parameter("out", [2, 128, NIMG * IMGE], BF16,
                                     isOutput=True)

    with tile.TileContext(nc) as tc:
        with (
            tc.tile_pool(name="big", bufs=1) as big,
            tc.tile_pool(name="wstage", bufs=2) as wsp,
            tc.tile_pool(name="wsc", bufs=3) as wscp,
            tc.tile_pool(name="xb", bufs=4) as xbp,
            tc.tile_pool(name="t2", bufs=3) as t2p,
            tc.tile_pool(name="ub", bufs=3) as ubp,
            tc.tile_pool(name="psum", bufs=6, space="PSUM") as psp,
            tc.tile_pool(name="pst", bufs=2, space="PSUM") as pstp,
        ):
            s1 = [big.tile([128, 2, SLEN], F8, tag=f"s1_{i}", name=f"s1_{i}")
                  for i in range(NIMG)]
            s2 = [big.tile([128, 2, SLEN], F8, tag=f"s2_{i}", name=f"s2_{i}")
                  for i in range(NIMG)]
            o1 = [[big.tile([128, IMGE], F32, tag=f"o1_{i}_{m}", name=f"o1_{i}_{m}")
                   for m in range(2)] for i in range(NIMG)]
            # conv1 m0 weights live in 3 per-tap-triple tiles so the first
            # matmuls only gate on the first 0.2MB chunk's DMA + binarize
            w1m0c = [big.tile([128, 3, 2, 128], F8, tag=f"w1m0c_{c}",
                              name=f"w1m0c_{c}") for c in range(3)]
            wf = {1: [None,
                      big.tile([128, 9, 2, 128], F8, tag="w1f_1",
                               name="w1f_1")],
                  2: [big.tile([128, 9, 2, 128], F8, tag=f"w2f_{m}",
                               name=f"w2f_{m}") for m in range(2)]}

            def lhsT_ap(convno, m, tap):
                if convno == 1 and m == 0:
                    return w1m0c[tap // 3][:, tap % 3, :, :]
                return wf[convno][m][:, tap, :, :]
            pt = big.tile([128, 2 * NPARAM], F32, tag="pt")
            dv = big.tile([128, 12], F32, tag="dv")   # per conv: sc, g*b, d
            scr = big.tile([128, 8], F32, tag="scr")
            scr2 = big.tile([128, 8], F32, tag="scr2")
            sc_g = big.tile([128, 4], F32, tag="sc_g")

            # ---- startup ----------------------------------------------
            # params DMA on the sync ring; bf16 sign-only copy of image 0
            # triggered from ScalarE so its transfer runs in parallel with
            # the w1-chunk transfers issued from the gpsimd ring.
            nc.sync.dma_start(out=pt[:, :], in_=pv[:, :])
            x0bt = big.tile([128, 2, IMGE], BF16, tag="x0bt")
            for h in range(2):
                nc.scalar.dma_start(out=x0bt[:, h, :], in_=x0b[h, :, :])
            nc.scalar.sign(out=scr2[:, 6:7],
                           in_=nc.const_aps.tensor(0.0, (128, 1)))
            # PE p-state warm-up: ~10 junk matmuls on zeroed tiles, gated
            # only on two cheap memsets (first in the vector queue), so the
            # PE array approaches max clock (~3us of continuous activity)
            # and stays busy until the first real matmul is ready.
            wz = big.tile([128, 2, 128], F8, tag="wz")
            dz = big.tile([128, 2, 576], F8, tag="dz")
            nc.vector.memset(wz[:, :, :].bitcast(U32), 0)
            nc.vector.memset(dz[:, :, :].bitcast(U32), 0)
            dzap = dz[:, :, :528].rearrange("p i (r c) -> p i r c", c=NCOL)
            dzap = dzap[:, :, :, :32]
            for k in range(10):
                psd = psp.tile([128, 512], F32, tag="ps", name="psd")
                nc.tensor.matmul(psd[:, :], wz[:, :, :], dzap,
                                 start=True, stop=True, perf_mode=DR)
            nc.vector.memset(s1[0][:, :, :].bitcast(U32), 0)
            nc.vector.memset(s1[1][:, :, :].bitcast(U32), 0)

            # w1[m0] in 3 tap-triple chunks (gpsimd ring), binarized on
            # VectorE (NEVER GpSimd: its ALU is ~18x slower and throttles
            # concurrent VectorE ops).
            wsc = []
            for c in range(3):
                ws = wscp.tile([128, 3, 2, 128], BF16, tag="wsc", name="ws")
                nc.gpsimd.dma_start(out=ws[:, :, :, :],
                                    in_=w1[0, :, 3 * c:3 * c + 3, :, :])
                wsc.append(ws)
            # sign(img0) from the bf16 copy, one op per half (before the
            # binarizes on the vector queue: it gates the first matmul)
            for h in range(2):
                sap = x0bt[:, h, :].rearrange("p (r c) -> p r c", c=32)
                dst = s1[0][:, h, GF:GF + PLANE]
                dst = dst.rearrange("p (r c) -> p r c", c=NCOL)
                nc.vector.tensor_scalar(dst[:, :32, :32], sap, 0.0, 0.5,
                                        AOP.is_gt, AOP.subtract)
            for c in range(3):
                nc.vector.tensor_scalar(w1m0c[c][:, :, :, :],
                                        wsc[c][:, :, :, :], 0.0, 0.5,
                                        AOP.is_gt, AOP.subtract)

            # serialization gate on the scalar queue: a 1-elem read of x0bt
            # completes only once both x0b DMAs landed, so the DMA triggers
            # queued after it cannot steal HBM bandwidth from the
            # startup-critical transfers.
            nc.scalar.activation(scr2[:, 7:8], x0bt[:, 0, 0:1], AFT.Identity)

            def dma_w(convno, m):
                src = w1 if convno == 1 else w2
                ws = wsp.tile([128, 9, 2, 128], BF16, tag="ws", name="ws")
                nc.scalar.dma_start(out=ws[:, :, :, :], in_=src[m, :, :, :, :])
                return ws

            def bin_w(convno, m, ws):
                nc.vector.tensor_scalar(wf[convno][m][:, :, :, :],
                                        ws[:, :, :, :], 0.0, 0.5,
                                        AOP.is_gt, AOP.subtract)

            xbt = {}

            def feed_dma(i, eng):
                """Two per-half DMAs for image i (two triggers -> the
                transfer spreads across DMA queues and finishes fast)."""
                xb = xbp.tile([128, 2, IMGE], F32, tag="xb", name="xb")
                for h in range(2):
                    eng.dma_start(out=xb[:, h, :],
                                  in_=xs[h, :, i * IMGE:(i + 1) * IMGE])
                xbt[i] = xb

            def feed_sign(i):
                nc.vector.memset(s1[i][:, :, :].bitcast(U32), 0)
                xr = xbt[i].rearrange("p i (r c) -> p i r c", c=32)
                dst = s1[i][:, :, GF:GF + PLANE]
                dst = dst.rearrange("p i (r c) -> p i r c", c=NCOL)
                nc.vector.tensor_scalar(dst[:, :, :32, :32], xr, 0.0, 0.5,
                                        AOP.is_gt, AOP.subtract)

            # priority order behind the gate: x1 (needed first), then --
            # behind a second gate so they cannot slow x1 down -- w1[m1] and
            # the f32 copy of img0 (only needed by img0's postops, which the
            # PSUM pool slack can wait for)
            # priority order behind the gate: x1 (needed first), then --
            # behind a second gate so they cannot slow x1 down -- w1[m1] and
            # the f32 copy of img0 (only needed by img0's postops, which the
            # PSUM pool slack can wait for)
            feed_dma(1, nc.scalar)          # sign of img1 emitted post-params
            nc.scalar.activation(scr2[:, 7:8], xbt[1][:, 0, 0:1],
                                 AFT.Identity)
            ws_w1m1 = dma_w(1, 1)
            feed_dma(0, nc.scalar)          # f32 img0 for the residual only

            # ---- folded BN/PReLU params (VectorE, Quake rsqrt) --------
            def pcol(m, name):
                k = PARAM_ORDER.index(name)
                return pt[:, m * NPARAM + k: m * NPARAM + k + 1]

            def dcol(j):
                return dv[:, j: j + 1]

            vco = [("bn1", 0), ("bn1", 1), ("bn2", 0), ("bn2", 1)]
            vpe = scr[:, 0:4]
            for j, (pfx, m) in enumerate(vco):
                nc.vector.tensor_scalar_add(scr[:, j:j + 1],
                                            pcol(m, pfx + "_var"), EPS)
            yb = scr[:, 4:8]
            nc.vector.memset(yb.bitcast(U32), 0x5f3759df)
            nc.vector.tensor_scalar(scr2[:, 0:4].bitcast(U32),
                                    vpe.bitcast(U32), 1, None,
                                    AOP.logical_shift_right)
            nc.vector.tensor_tensor(yb.bitcast(U32), yb.bitcast(U32),
                                    scr2[:, 0:4].bitcast(U32), AOP.subtract)
            for _ in range(3):
                nc.vector.tensor_tensor(scr2[:, 0:4], yb, yb, AOP.mult)
                nc.vector.tensor_tensor(scr2[:, 0:4], vpe, scr2[:, 0:4],
                                        AOP.mult)
                nc.vector.tensor_scalar(scr2[:, 0:4], scr2[:, 0:4], -0.5, 1.5,
                                        AOP.mult, AOP.add)
                nc.vector.tensor_tensor(yb, yb, scr2[:, 0:4], AOP.mult)

            for j, (pfx, m) in enumerate(vco):
                ci = j // 2
                gam = pcol(m, pfx + "_gamma")
                bet = pcol(m, pfx + "_beta")
                mean = pcol(m, pfx + "_mean")
                bvec = pcol(m, "b1" if ci == 0 else "b2")
                rs = yb[:, j:j + 1]
                g = scr2[:, 4:5]
                nc.vector.tensor_tensor(g, gam, rs, AOP.mult)
                # conv1: acts +-0.5, w +-0.5 -> x4; conv2: acts +-1 -> x2
                nc.vector.tensor_scalar_mul(dcol(ci * 6 + m), g,
                                            STD * (4.0 if ci == 0 else 2.0))
                nc.vector.tensor_tensor(dcol(ci * 6 + 2 + m), g, bvec,
                                        AOP.mult)
                nc.vector.tensor_tensor(scr2[:, 5:6], mean, g, AOP.mult)
                nc.vector.tensor_tensor(dcol(ci * 6 + 4 + m), bet,
                                        scr2[:, 5:6], AOP.subtract)

            def sc_ap(conv, m):
                return dcol((conv - 1) * 6 + m)

            def bi_ap(conv, m):
                return dcol((conv - 1) * 6 + 2 + m)

            def dd_ap(conv, m):
                return dcol((conv - 1) * 6 + 4 + m)

            def al_ap(m):
                return pcol(m, "alpha")

            # post-params vector work: s2 memsets, sign(img1), w1[m1] binarize
            nc.vector.memset(s2[0][:, :, :].bitcast(U32), 0)
            nc.vector.memset(s2[1][:, :, :].bitcast(U32), 0)
            xr1 = xbt[1].rearrange("p i (r c) -> p i r c", c=32)
            dst1 = s1[1][:, :, GF:GF + PLANE]
            dst1 = dst1.rearrange("p i (r c) -> p i r c", c=NCOL)
            nc.vector.tensor_scalar(dst1[:, :, :32, :32], xr1, 0.0, 0.5,
                                    AOP.is_gt, AOP.subtract)
            bin_w(1, 1, ws_w1m1)

            def rhs_ap(st, r0, nr, tap):
                dy, dx = divmod(tap, 3)
                off = (dy - 1) * NCOL + (dx - 1)
                base = GF + r0 * NCOL + off
                ap = st[:, :, base: base + nr * NCOL]
                ap = ap.rearrange("p i (r c) -> p i r c", c=NCOL)
                return ap[:, :, :, :32]

            def postops(convno, img, m, ps_list, e0, ln):
                """Evac ps_list (consecutive windows covering [e0, e0+ln) of
                the image) + PReLU + residual-add (+ sign / output DMA)."""
                first = convno == 1
                if first:
                    dst = o1[img][m][:, e0:e0 + ln]
                else:
                    tb = t2p.tile([128, ln], F32, tag="t2", name="t2")
                    dst = tb[:, :]
                off = 0
                for ps in ps_list:
                    _ln = ps.shape[1]
                    # fused evac: t = Prelu(S*sc + g*b; alpha) on ScalarE
                    nc.scalar.activation(dst[:, off:off + _ln], ps[:, :],
                                         AFT.Prelu,
                                         bias=bi_ap(convno, m),
                                         scale=sc_ap(convno, m),
                                         alpha=al_ap(m))
                    off += _ln
                if first:
                    # out1 = (t + d1) + x ; sign -> s2 interior
                    nc.vector.scalar_tensor_tensor(
                        dst, dst, dd_ap(1, m),
                        xbt[img][:, m, e0:e0 + ln], AOP.add, AOP.add)
                    sdst = s2[img][:, m, GF:GF + PLANE]
                    sdst = sdst.rearrange("p (r c) -> p r c", c=NCOL)
                    nc.scalar.sign(
                        out=sdst[:, e0 // 32:(e0 + ln) // 32, :32],
                        in_=dst.rearrange("p (r c) -> p r c", c=32))
                else:
                    ub = ubp.tile([128, ln], BF16, tag="ub", name="ub")
                    nc.vector.scalar_tensor_tensor(
                        ub[:, :], dst, dd_ap(2, m),
                        o1[img][m][:, e0:e0 + ln], AOP.add, AOP.add)
                    nc.sync.dma_start(
                        out=outd[m, :, img * IMGE + e0:img * IMGE + e0 + ln],
                        in_=ub[:, :])

            def conv_img(convno, img, tail=False):
                """All matmuls + postops for one image of one conv."""
                first = convno == 1
                st = (s1 if first else s2)[img]
                for m in range(2):
                    if tail and m == 1:
                        # shrink the final drain: last windows are 8 rows
                        wins = [(0, 16, psp), (16, 8, pstp), (24, 8, pstp)]
                    else:
                        wins = [(0, 16, psp), (16, 16, psp)]
                    ps = [pool.tile([128, nr * 32], F32, tag="ps", name="ps")
                          for _, nr, pool in wins]
                    for tap in range(9):
                        for wi, (r0, nr, _) in enumerate(wins):
                            nc.tensor.matmul(ps[wi][:, :],
                                             lhsT_ap(convno, m, tap),
                                             rhs_ap(st, r0, nr, tap),
                                             start=(tap == 0),
                                             stop=(tap == 8),
                                             perf_mode=DR)
                    if tail and m == 1:
                        for wi, (r0, nr, _) in enumerate(wins):
                            postops(convno, img, m, [ps[wi]], r0 * 32,
                                    nr * 32)
                    else:
                        postops(convno, img, m, ps, 0, IMGE)

            # ---- conv1 over images --------------------------------------
            # Feeds 2..7 ride a DMA-completion chain on the sync ring: each
            # trigger is preceded by a 1-element gate DMA that reads the
            # chain's previous buffer, so its issue waits for that transfer
            # to finish.  Transfers therefore run strictly one at a time in
            # priority order (never sharing HBM bandwidth with an urgent
            # one), and the pacing never depends on downstream compute.
            ws_w2 = {}
            for img in range(NIMG):
                conv_img(1, img)
                if img + 2 < NIMG:
                    feed_dma(img + 2, nc.scalar)
                    feed_sign(img + 2)
                    nc.vector.memset(s2[img + 2][:, :, :].bitcast(U32), 0)
                if img == 0:
                    ws_w2[0] = dma_w(2, 0)
                    bin_w(2, 0, ws_w2[0])
                elif img == 1:
                    ws_w2[1] = dma_w(2, 1)
                    bin_w(2, 1, ws_w2[1])

            # ---- conv2 ------------------------------------------------
            for img in range(NIMG):
                conv_img(2, img, tail=(img == NIMG - 1))

    nc.compile()
    return nc


# ---------------------------------------------------------------- host side

def _host_pack_x(x_shard):
    """[8,256,32,32] f32 -> [2,128,8192] f32 compact."""
    xr = np.asarray(x_shard, np.float32).reshape(NIMG, 2, 128, IMGE)
    return np.ascontiguousarray(xr.transpose(1, 2, 0, 3)
                                .reshape(2, 128, NIMG * IMGE))


def _host_pack_w(W):
    """[256,256,3,3] -> [2(m), 128(k), 9(tap), 2(i), 128(j)] bf16."""
    A = np.asarray(W, np.float32).reshape(2, 128, 2, 128, 3, 3)
    L = A.transpose(0, 3, 4, 5, 2, 1)          # (m, k, dy, dx, i, j)
    L = np.ascontiguousarray(L.reshape(2, 128, 9, 2, 128))
    return L.astype(ml_dtypes.bfloat16)


def _host_pack_pv(inputs):
    pvt = np.zeros((128, 2 * NPARAM), dtype=np.float32)
    for k, name in enumerate(PARAM_ORDER):
        v = np.asarray(inputs[name], dtype=np.float32)
        for m in range(2):
            pvt[:, m * NPARAM + k] = v[m * 128:(m + 1) * 128]
    return pvt


def _host_unpack_out(o):
    """[2,128,8192] bf16 -> [8,256,32,32] f32."""
    o = np.asarray(o, dtype=np.float32).reshape(2, 128, NIMG, 32, 32)
    return np.ascontiguousarray(o.transpose(2, 0, 1, 3, 4)
                                .reshape(NIMG, 256, 32, 32))


_PROG = None
LAST_EXEC_TIME_NS = None


def _get_prog():
    global _PROG
    if _PROG is None:
        _PROG = build_program()
    return _PROG


def kernel(x, W1, b1, W2, b2, alpha,
           bn1_gamma, bn1_beta, bn1_mean, bn1_var,
           bn2_gamma, bn2_beta, bn2_mean, bn2_var,
           _trace=False):
    global LAST_EXEC_TIME_NS
    inputs = dict(b1=b1, b2=b2, alpha=alpha,
                  bn1_gamma=bn1_gamma, bn1_beta=bn1_beta,
                  bn1_mean=bn1_mean, bn1_var=bn1_var,
                  bn2_gamma=bn2_gamma, bn2_beta=bn2_beta,
                  bn2_mean=bn2_mean, bn2_var=bn2_var)
    x = np.asarray(x, dtype=np.float32)
    w1l = _host_pack_w(W1)
    w2l = _host_pack_w(W2)
    pvt = _host_pack_pv(inputs)

    in_maps = []
    for c in range(NCORES):
        shard = x[c * NIMG:(c + 1) * NIMG]
        xp = _host_pack_x(shard)
        in_maps.append({"xs": xp, "w1": w1l, "w2": w2l, "pv": pvt,
                        "x0b": np.ascontiguousarray(xp[:, :, :IMGE])
                        .astype(ml_dtypes.bfloat16)})

    nc = _get_prog()
    res = run_bass_kernel_spmd(nc, in_maps, core_ids=list(range(NCORES)),
                               trace=_trace)
    LAST_EXEC_TIME_NS = res.exec_time_ns

    outs = [_host_unpack_out(res.results[c]["out"]) for c in range(NCORES)]
    return np.concatenate(outs, axis=0)
